# revision 1
# baseline (speedup 1.0000x reference)
"""Trainium2 Bass kernel for nn_Block_35880156790920 (dense transformer block).

Sharding: 8 cores = 2 batches x 4 query-token-blocks (data parallel on B and
S). Each core computes the full block output for its 512-token slice; K/V
projections for the whole batch are computed redundantly per core (no
collectives needed).

Per-core pipeline (all matmuls bf16 operands, fp32 accumulate):
  LN1 (token-major, fp32 stats) -> xn1 bf16 -> DMA-transpose -> xn1T
  QKV proj (PE; bias rows added via K=1 matmuls into PSUM)
  l2norm(q)*exp(clamped logit_scale), l2norm(k)  (token-major)
  DMA-transpose qn,kn -> feature-major; v kept token-major with ones column
  flash-style attention per head pair: scoresT -> exp (ACT, 2 heads/op)
    -> ctxT + softmax denominator via [v|1] matmul accumulation
  normalize ctx (batched reciprocal + ones-matmul row broadcast),
  out-proj + residual, LN2, MLP (gelu bias fused into ACT), residual -> y.
"""

from contextlib import ExitStack

import numpy as np
import ml_dtypes

import concourse.bass as bass
import concourse.tile as tile
from concourse import bacc, mybir
from concourse.bass import ts, ds
from concourse.bass_utils import run_bass_kernel_spmd

F32 = mybir.dt.float32
F32R = mybir.dt.float32r
BF16 = mybir.dt.bfloat16
AF = mybir.ActivationFunctionType
ALU = mybir.AluOpType

P = 128
B, S, D = 2, 2048, 1024
H, HD = 16, 64
MLP = 4096
SQ = S // 4          # 512 query tokens per core
DC = D // P          # 8
TB = S // P          # 16
TQ = SQ // P         # 4
MC = MLP // P        # 32
HP = H // 2          # 8 head pairs
EPS_LN = 1e-6
EPS_NORM = 1e-12
LOG_MAX = float(np.log(1.0 / 0.01))
N_CORES = 8
SKIP_CC = False

_CACHED_NC = {}


def _emit_once(tc, outs, ins, pools):
    nc = tc.nc

    xq, xqr = ins["xq"], ins["xqr"]
    y = outs["y"]

    # ---- constants ----
    eps_tile = pools["const"].tile([P, 1], F32, tag="eps", name="eps")
    nc.vector.memset(eps_tile[:], EPS_LN)
    eps0 = pools["const"].tile([P, 1], F32, tag="eps0", name="eps0")
    nc.vector.memset(eps0[:], 0.0)
    ones_tok = pools["const"].tile([1, P], BF16, tag="ones_tok", name="ones_tok")
    nc.vector.memset(ones_tok[:], 1.0)
    ones_hd = pools["const"].tile([1, HD], F32, tag="ones_hd", name="ones_hd")  # fp32 for the fp32 bcast matmul
    nc.vector.memset(ones_hd[:], 1.0)

    # bias rows live in DRAM; streamed per-use into [1,512] scratch tiles
    def bias_rhs(name, n):
        rrow = pools["rrow"].tile([1, 512], BF16, tag="rrow", name="rrow")
        nc.sync.dma_start(rrow[:], ins[name][0:1, ts(n, 512)])
        return rrow[:]

    bias_m = pools["const"].tile([P, MC], F32, tag="bias_m", name="bias_m")
    nc.sync.dma_start(bias_m[:], ins["bias_m"][:])

    # per-head scale c = exp(min(logit_scale, LOG_MAX)), broadcast on partitions
    crow = pools["const"].tile([1, H], F32, tag="crow", name="crow")
    nc.sync.dma_start(crow[:], ins["ck"][:])
    c_b = pools["const"].tile([P, H], F32, tag="c_b", name="c_b")
    nc.gpsimd.partition_broadcast(c_b[:], crow[:])

    # ---- persistent activations ----
    xnqT = pools["xnqT"].tile([P, DC, SQ], BF16, tag="xnqT", name="xnqT")   # 1 MB
    knT = pools["knT"].tile([P, DC, S], BF16, tag="knT", name="knT")       # 4 MB
    qnT = pools["qnT"].tile([P, DC, SQ], BF16, tag="qnT", name="qnT")      # 1 MB
    vaug = pools["vaug"].tile([P, TB, H, HD + 1], BF16, tag="vaug", name="vaug")  # 4.25 MB
    ao_dram = pools["dram"].tile([SQ, D], F32, tag="aodram", name="aodram")
    knTo = pools["ctxU"].tile([P, DC, SQ], BF16, tag="ctxU", name="knTo")
    vaugo = pools["ctxU"].tile([P, TQ, H, HD + 1], BF16, tag="btmp", name="vaugo")
    den_halves = [
        pools["den"].tile([H // 2, SQ], F32, tag="den_lo", name="den_lo"),
        pools["den"].tile([H // 2, SQ], F32, tag="den_hi", name="den_hi"),
    ]

    def den_row(h):
        return den_halves[h // (H // 2)][h % (H // 2):h % (H // 2) + 1, :]

    # ones columns of own v-augmented (v evictions later overwrite cols 0:HD)
    nc.vector.memset(vaugo[:], 1.0)

    def ln_tile(x_ap, out_bf16_ap):
        """LayerNorm stats+apply for one [P, D] fp32 tile -> bf16 (gain folded
        into weights on host, ln-bias folded into projection bias rows)."""
        st = pools["stats"].tile([P, 2, 6], F32, tag="st", name="st")
        xr = x_ap.rearrange("p (s d) -> p s d", s=2)
        for i in range(2):
            nc.vector.bn_stats(st[:, i, :], xr[:, i, :])
        mv = pools["stats"].tile([P, 2], F32, tag="mv", name="mv")
        nc.vector.bn_aggr(mv[:], st[:])
        rstd = pools["stats"].tile([P, 1], F32, tag="rstd", name="rstd")
        nc.scalar.activation(rstd[:], mv[:, 1:2], AF.Sqrt, bias=eps_tile[:])
        nc.vector.reciprocal(rstd[:], rstd[:])
        nc.vector.tensor_scalar(out_bf16_ap, x_ap, scalar1=mv[:, 0:1],
                                scalar2=rstd[:], op0=ALU.subtract, op1=ALU.mult)

    # ---- PE warm-up: keep HAM busy while LN1 runs (results unused but kept
    # live via a tiny DRAM spill so DCE keeps them) ----
    wu = pools["const"].tile([P, P], BF16, tag="wu", name="wu")
    nc.vector.memset(wu[:], 0.5)
    wups = pools["mm512"].tile([P, 512], F32, tag="mm512", name="wups")
    for i in range(40):
        nc.tensor.matmul(wups[:, 0:P], wu[:], wu[:],
                         start=(i == 0), stop=(i == 39), skip_group_check=True)
    wusb = pools["const"].tile([P, 4], F32, tag="wusb", name="wusb")
    nc.vector.tensor_copy(wusb[:], wups[:, 0:4])
    wuspill = pools["dram"].tile([P, 4], F32, tag="wuspill", name="wuspill")
    nc.sync.dma_start(wuspill[:], wusb[:])

    # ---- LN1 over own tokens -> xnqT ----
    for t in range(TQ):
        x_t = pools["xin"].tile([P, D], F32, tag="x", name="x")
        nc.sync.dma_start(x_t[:], xq[ts(t, P), :])
        xn_t = pools["xn"].tile([P, D], BF16, tag="xn", name="xn")
        ln_tile(x_t[:], xn_t[:])
        for d in range(DC):
            nc.sync.dma_start(xnqT[:, d, ts(t, P)], xn_t[:, ts(d, P)], transpose=True)

    # ---- QKV projections ----
    def l2norm_scale_transpose(t, kq_t, dstT, scale_pp):
        """kq_t: [P, D] bf16 token-major; optional scale_pp [P, H] extra
        multiplier; writes l2-normalized transpose into dstT[:, :, ts(t, P)]."""
        sq = pools["sq"].tile([P, D], BF16, tag="sq", name="sq")
        nc.scalar.activation(sq[:], kq_t[:], AF.Square)
        ss = pools["stats"].tile([P, H], F32, tag="ss", name="ss")
        nc.vector.tensor_reduce(ss[:], sq[:].rearrange("p (h d) -> p h d", h=H),
                                axis=mybir.AxisListType.X, op=ALU.add)
        nrm = pools["stats"].tile([P, H], F32, tag="nrm", name="nrm")
        nc.scalar.activation(nrm[:], ss[:], AF.Sqrt, bias=eps0[:])
        nc.vector.tensor_scalar_max(nrm[:], nrm[:], EPS_NORM)
        rinv = pools["stats"].tile([P, H], F32, tag="rinv", name="rinv")
        nc.vector.reciprocal(rinv[:], nrm[:])
        if scale_pp is not None:
            nc.vector.tensor_tensor(rinv[:], rinv[:], scale_pp, op=ALU.mult)
        kn_t = pools["xn"].tile([P, D], BF16, tag="xn", name="xn")
        nc.vector.tensor_tensor(
            kn_t[:].rearrange("p (h d) -> p h d", h=H),
            kq_t[:].rearrange("p (h d) -> p h d", h=H),
            rinv[:, :, None].broadcast_to([P, H, HD]), op=ALU.mult)
        for d in range(DC):
            nc.sync.dma_start(dstT[:, d, ts(t, P)], kn_t[:, ts(d, P)], transpose=True)

    qk_tiles = {}

    def evict_q(t, ps):
        q_t = pools["qk"].tile([P, D], BF16, tag="qk", name="qk")
        qk_tiles[t] = q_t
        nc.vector.tensor_copy(q_t[:], ps[:])
        l2norm_scale_transpose(t, q_t, qnT, c_b[:])

    def evict_k(t, ps):
        k_t = pools["qk"].tile([P, D], BF16, tag="qk", name="qk")
        qk_tiles[t] = k_t
        nc.vector.tensor_copy(k_t[:], ps[:])
        l2norm_scale_transpose(t, k_t, knTo, None)

    def evict_v(t, ps):
        nc.vector.tensor_copy(vaugo[:, t, :, 0:HD],
                              ps[:].rearrange("p (h d) -> p h d", h=H))

    def proj(w_name, bias_name, src_T, ntiles, evict):
        w_sb = pools["w"].tile([P, DC, D], BF16, tag="w", name="w")
        nc.sync.dma_start(
            w_sb[:], ins[w_name][:].rearrange("(dc p) c -> p dc c", p=P))
        for t in range(ntiles):
            ps = pools["score"].tile([P, 1024], F32, tag="score", name="psqkv")
            for d in range(DC):
                lhs = src_T[:, d, ts(t, P)]
                nc.tensor.matmul(ps[:, 0:512], lhs, w_sb[:, d, 0:512],
                                 start=(d == 0), stop=False,
                                 skip_group_check=True)
                nc.tensor.matmul(ps[:, 512:1024], lhs, w_sb[:, d, 512:1024],
                                 start=(d == 0), stop=False,
                                 skip_group_check=True)
            for n in range(2):
                nc.tensor.matmul(ps[:, ts(n, 512)], ones_tok[:],
                                 bias_rhs(bias_name, n),
                                 start=False, stop=True, skip_group_check=True)
            evict(t, ps)

    KVK = DC * SQ
    KVV = TQ * H * (HD + 1)
    GROUPS = [[0, 1, 2, 3], [4, 5, 6, 7]]

    # K projection, then its gather starts while V/Q projections run
    proj("wk", "bias_k", xnqT, TQ, evict_k)
    kb = pools["dram"].tile([P, KVK], BF16, tag="kb", name="kb")
    kg = pools["dram"].tile([4, P, KVK], BF16, tag="kg", name="kg")
    nc.sync.dma_start(kb[:], knTo[:].rearrange("p d s -> p (d s)"))
    if SKIP_CC:
        for g in range(4):
            nc.sync.dma_start(kg[g], kb[:])
    else:
        nc.gpsimd.collective_compute(
            "AllGather", ALU.bypass, replica_groups=GROUPS,
            ins=[kb[:].opt()], outs=[kg[:].opt()])

    proj("wv", "bias_v", xnqT, TQ, evict_v)
    vb = pools["dram"].tile([P, KVV], BF16, tag="vb", name="vb")
    vg = pools["dram"].tile([4, P, KVV], BF16, tag="vg", name="vg")
    nc.sync.dma_start(vb[:], vaugo[:].rearrange("p t h d -> p (t h d)"))
    if SKIP_CC:
        for g in range(4):
            nc.sync.dma_start(vg[g], vb[:])
    else:
        nc.gpsimd.collective_compute(
            "AllGather", ALU.bypass, replica_groups=GROUPS,
            ins=[vb[:].opt()], outs=[vg[:].opt()])

    # q projection runs while the collectives are in flight
    proj("wq", "bias_q", xnqT, TQ, evict_q)
    for g in range(4):
        for d in range(DC):
            nc.sync.dma_start(knT[:, d, ds(SQ * g, SQ)],
                              kg[g, :, ds(512 * d, 512)])
        nc.sync.dma_start(
            vaug[:, ds(TQ * g, TQ), :, :],
            vg[g].rearrange("p (t h d) -> p t h d", t=TQ, h=H))

    # ---- attention: head pairs ----
    ctxU = pools["ctxU"].tile([P, DC, SQ], BF16, tag="ctxU", name="ctxU")
    btmp = pools["ctxU"].tile([HD, HP, SQ], BF16, tag="btmp", name="btmp")
    # softmax denominators: half-batched reciprocal + K=1 broadcast matmul,
    # emitted per 4-head-pair wave so normalization overlaps later attention
    def normalize_heads(h0, h1):
        dh = den_halves[h0 // (H // 2)]
        nc.vector.reciprocal(dh[:], dh[:])
        for h in range(h0, h1):
            hp = h // 2
            rd0 = pools["rd0"].tile([1, SQ], F32, tag="rd0", name="rd0")
            nc.sync.dma_start(rd0[:], den_row(h))
            dn = pools["mm512"].tile([P, 512], F32, tag="mm512", name="mm512")
            nc.tensor.matmul(dn[0:HD, :], ones_hd[:], rd0[:],
                             start=True, stop=True)
            if h % 2 == 0:
                nc.vector.tensor_tensor(ctxU[0:HD, hp, :], ctxU[0:HD, hp, :],
                                        dn[0:HD, :], op=ALU.mult)
            else:
                nc.vector.tensor_tensor(btmp[:, hp, :], btmp[:, hp, :],
                                        dn[0:HD, :], op=ALU.mult)
                nc.sync.dma_start(ctxU[HD:P, hp, :], btmp[:, hp, :])


    for hp in range(HP):
        hA, hB = 2 * hp, 2 * hp + 1
        # alternate psum pools so the next pair's accumulators don't wait on
        # this pair's evictions (mm512 banks are idle during the hp loop)
        cpool = pools["ctx"] if hp % 2 == 0 else pools["mm512"]
        ctag = "ctx" if hp % 2 == 0 else "mm512"
        ctxA = cpool.tile([HD + 1, 512], F32, tag=ctag, name="ctx")
        ctxB = cpool.tile([HD + 1, 512], F32, tag=ctag, name="ctx")
        def emit_scores(kt):
            sc = pools["score"].tile([P, 1024], F32, tag="score", name="score")
            nc.tensor.matmul(sc[:, 0:512], knT[0:HD, hp, ts(kt, P)],
                             qnT[0:HD, hp, :], start=True, stop=True,
                             tile_position=(0, 0), skip_group_check=True)
            nc.tensor.matmul(sc[:, 512:1024], knT[HD:P, hp, ts(kt, P)],
                             qnT[HD:P, hp, :], start=True, stop=True,
                             tile_position=(64, 0), skip_group_check=True)
            return sc

        # software pipeline: kt+1's scores issue on the PE before kt's ctx
        # matmuls, so the in-order PE never stalls waiting for exp(kt)
        sc = emit_scores(0)
        for kt in range(TB):
            eT = pools["eT"].tile([P, 1024], BF16, tag="eT", name="eT")
            nc.scalar.activation(eT[:], sc[:], AF.Exp)
            if kt + 1 < TB:
                sc = emit_scores(kt + 1)
            nc.tensor.matmul(ctxA[:], vaug[:, kt, hA, :], eT[:, 0:512],
                             start=(kt == 0), stop=(kt == TB - 1),
                             skip_group_check=True)
            nc.tensor.matmul(ctxB[:], vaug[:, kt, hB, :], eT[:, 512:1024],
                             start=(kt == 0), stop=(kt == TB - 1),
                             skip_group_check=True)
        # unnormalized evictions + denominator collection
        nc.vector.tensor_copy(ctxU[0:HD, hp, :], ctxA[0:HD, :])
        nc.vector.tensor_copy(btmp[:, hp, :], ctxB[0:HD, :])
        dtmp = pools["dtmp"].tile([HD + 1, 2, 512], F32, tag="dtmp", name="dtmp")
        nc.vector.tensor_copy(dtmp[HD:HD + 1, 0, :], ctxA[HD:HD + 1, :])
        nc.vector.tensor_copy(dtmp[HD:HD + 1, 1, :], ctxB[HD:HD + 1, :])
        nc.sync.dma_start(den_row(hA), dtmp[HD:HD + 1, 0, :])
        nc.sync.dma_start(den_row(hB), dtmp[HD:HD + 1, 1, :])
        if hp == HP // 2 - 1:
            normalize_heads(0, H // 2)
        elif hp == HP - 1:
            normalize_heads(H // 2, H)

    # ---- out-projection + residual -> ao (fp32, token-major) ----
    wo_sb = pools["w"].tile([P, DC, D], BF16, tag="w", name="w")
    nc.sync.dma_start(wo_sb[:], ins["wo"][:].rearrange("(dc p) c -> p dc c", p=P))
    xn2T = pools["xnqT"].tile([P, DC, SQ], BF16, tag="xnqT", name="xn2T")
    for t in range(TQ):
        ao_t = pools["ao"].tile([P, D], F32, tag="ao", name="ao")
        ps = pools["score"].tile([P, 1024], F32, tag="score", name="psao")
        for d in range(DC):
            lhs = ctxU[:, d, ts(t, P)]
            nc.tensor.matmul(ps[:, 0:512], lhs, wo_sb[:, d, 0:512],
                             start=(d == 0), stop=(d == DC - 1),
                             skip_group_check=True)
            nc.tensor.matmul(ps[:, 512:1024], lhs, wo_sb[:, d, 512:1024],
                             start=(d == 0), stop=(d == DC - 1),
                             skip_group_check=True)
        for n in range(2):
            xqr_t = pools["xqr"].tile([P, 512], F32, tag="xqr", name="xqr")
            nc.sync.dma_start(xqr_t[:], xqr[ts(t, P), ts(n, 512)])
            nc.vector.tensor_tensor(ao_t[:, ts(n, 512)], ps[:, ts(n, 512)],
                                    xqr_t[:], op=ALU.add)
        # LN2 for this tile + spill ao to DRAM for the fc2 residual
        xn_t = pools["xn"].tile([P, D], BF16, tag="xn", name="xn")
        ln_tile(ao_t[:], xn_t[:])
        for d in range(DC):
            nc.sync.dma_start(xn2T[:, d, ts(t, P)], xn_t[:, ts(d, P)], transpose=True)
        nc.sync.dma_start(ao_dram[ts(t, P), :], ao_t[:])

    # ---- MLP fc1: h1T feature-major with fused gelu+bias ----
    h1gA = pools["xnT"].tile([P, MC // 2, SQ], BF16, tag="xnTa", name="h1gA")
    h1gB = pools["xnT"].tile([P, MC // 2, SQ], BF16, tag="xnTb", name="h1gB")

    def h1g(m):
        return h1gA[:, m, :] if m < MC // 2 else h1gB[:, m - MC // 2, :]

    for m in range(MC):
        w1_m = pools["w1"].tile([P, DC, P], BF16, tag="w1", name="w1")
        nc.sync.dma_start(
            w1_m[:], ins["w1"][:, ts(m, P)].rearrange("(dc p) c -> p dc c", p=P))
        ps = pools["mm512"].tile([P, 512], F32, tag="mm512", name="mm512")
        for d in range(DC):
            nc.tensor.matmul(ps[:], w1_m[:, d, :], xn2T[:, d, :],
                             start=(d == 0), stop=(d == DC - 1))
        nc.scalar.activation(h1g(m), ps[:], AF.Gelu,
                             bias=bias_m[:, m:m + 1])

    # ---- MLP fc2 + bias + residual -> y ----
    w2_tiles = [
        pools["knT"].tile([P, MC, 512], BF16, tag="knT", name="w2n0"),
        pools["vaug"].tile([P, MC, 512], BF16, tag="vaug", name="w2n1"),
    ]
    for n in range(2):
        w2_n = w2_tiles[n]
        nc.sync.dma_start(
            w2_n[:], ins["w2"][:, ts(n, 512)].rearrange("(mc p) c -> p mc c", p=P))
        for t in range(TQ):
            ps = pools["mm512"].tile([P, 512], F32, tag="mm512", name="mm512")
            for m in range(MC):
                nc.tensor.matmul(ps[:], h1g(m)[:, ts(t, P)], w2_n[:, m, :],
                                 start=(m == 0), stop=False)
            nc.tensor.matmul(ps[:], ones_tok[:], bias_rhs("b2", n),
                             start=False, stop=True)
            aor = pools["xqr"].tile([P, 512], F32, tag="xqr", name="aor")
            nc.sync.dma_start(aor[:], ao_dram[ts(t, P), ts(n, 512)])
            y_t = pools["yout"].tile([P, 512], F32, tag="yout", name="yout")
            nc.vector.tensor_tensor(y_t[:], ps[:], aor[:], op=ALU.add)
            nc.sync.dma_start(y[ts(t, P), ts(n, 512)], y_t[:])


def build_program(repeat=1, skip_cc=False):
    global SKIP_CC
    SKIP_CC = skip_cc
    nc = bacc.Bacc("TRN2", target_bir_lowering=False, debug=False)
    ins = {}

    def din(name, shape, dt=F32):
        ins[name] = nc.dram_tensor(name, list(shape), dt, kind="ExternalInput").ap()

    din("xq", [SQ, D]); din("xqr", [SQ, D])
    din("wq", [D, D], BF16); din("wk", [D, D], BF16); din("wv", [D, D], BF16)
    din("wo", [D, D], BF16)
    din("w1", [D, MLP], BF16); din("w2", [MLP, D], BF16)
    din("bias_q", [1, D], BF16); din("bias_k", [1, D], BF16)
    din("bias_v", [1, D], BF16)
    din("bias_m", [P, MC]); din("b2", [1, D], BF16); din("ck", [1, H])
    outs = {"y": nc.dram_tensor("y", [SQ, D], F32, kind="ExternalOutput").ap()}

    with tile.TileContext(nc) as tc:
        with ExitStack() as es:
            pools = {}

            def pool(name, bufs, space="SBUF"):
                pools[name] = es.enter_context(
                    tc.tile_pool(name=name, bufs=bufs, space=space))

            pool("const", 1)
            pool("xnT", 1); pool("xnqT", 1); pool("knT", 1); pool("qnT", 1)
            pool("vaug", 1); pool("xqr", 2); pool("ao", 2); pool("ctxU", 1)
            pool("den", 1); pool("rd0", 1); pool("dtmp", 1); pool("rrow", 2)
            pool("xin", 2); pool("xn", 3); pool("stats", 4); pool("sq", 1)
            pool("qk", 2); pool("w", 1); pool("w1", 2)
            pool("eT", 3); pool("yout", 1)
            pool("dram", 1, space="DRAM")
            pool("mm512", 2, space="PSUM")
            pool("score", 2, space="PSUM")
            pool("ctx", 2, space="PSUM")
            for _ in range(repeat):
                _emit_once(tc, outs, ins, pools)
    nc.compile()
    return nc


def _host_prep(inputs):
    """Host-side slicing + folding. Returns per-core in_maps."""
    f32 = np.float32
    bf16 = ml_dtypes.bfloat16
    x = np.asarray(inputs["x"], f32)
    ln1_g = np.asarray(inputs["ln1_g"], f32); ln1_b = np.asarray(inputs["ln1_b"], f32)
    ln2_g = np.asarray(inputs["ln2_g"], f32); ln2_b = np.asarray(inputs["ln2_b"], f32)
    wq = np.asarray(inputs["wq"], f32); wk = np.asarray(inputs["wk"], f32)
    wv = np.asarray(inputs["wv"], f32); wo = np.asarray(inputs["wo"], f32)
    w1 = np.asarray(inputs["w1"], f32); w2 = np.asarray(inputs["w2"], f32)
    bq = np.asarray(inputs["bq"], f32); bk = np.asarray(inputs["bk"], f32)
    bv = np.asarray(inputs["bv"], f32); bo = np.asarray(inputs["bo"], f32)
    b1 = np.asarray(inputs["b1"], f32); b2 = np.asarray(inputs["b2"], f32)
    ls = np.asarray(inputs["logit_scale"], f32).reshape(H)

    shared = dict(
        wq=(ln1_g[:, None] * wq).astype(bf16),
        wk=(ln1_g[:, None] * wk).astype(bf16),
        wv=(ln1_g[:, None] * wv).astype(bf16),
        wo=wo.astype(bf16),
        w1=(ln2_g[:, None] * w1).astype(bf16),
        w2=w2.astype(bf16),
        bias_q=(ln1_b @ wq + bq).astype(bf16).reshape(1, D),
        bias_k=(ln1_b @ wk + bk).astype(bf16).reshape(1, D),
        bias_v=(ln1_b @ wv + bv).astype(bf16).reshape(1, D),
        bias_m=(ln2_b @ w1 + b1).astype(f32).reshape(MC, P).T.copy(),
        b2=b2.astype(bf16).reshape(1, D),
        ck=np.exp(np.minimum(ls, LOG_MAX)).astype(f32).reshape(1, H),
    )
    in_maps = []
    for c in range(N_CORES):
        b = c // 4
        t = c % 4
        sl = slice(t * SQ, (t + 1) * SQ)
        m = dict(shared)
        m["xq"] = np.ascontiguousarray(x[b, sl])
        m["xqr"] = np.ascontiguousarray(x[b, sl] + bo[None, :])
        in_maps.append(m)
    return in_maps


def kernel(**inputs):
    if "main" not in _CACHED_NC:
        _CACHED_NC["main"] = build_program()
    nc = _CACHED_NC["main"]
    in_maps = _host_prep(inputs)
    res = run_bass_kernel_spmd(nc, in_maps, core_ids=list(range(N_CORES)))
    y = np.empty((B, S, D), np.float32)
    for c in range(N_CORES):
        b = c // 4
        t = c % 4
        y[b, t * SQ:(t + 1) * SQ] = res.results[c]["y"]
    return y



# revision 3
# speedup vs baseline: 1.4200x; 1.4200x over previous
"""Trainium2 Bass kernel for nn_Block_35880156790920 (dense transformer block).

Sharding: 8 cores = 2 batches x 4 query-token-blocks (data parallel on B and
S). Each core computes the full block output for its 512-token slice; K/V
projections for the whole batch are computed redundantly per core (no
collectives needed).

Per-core pipeline (all matmuls bf16 operands, fp32 accumulate):
  LN1 (token-major, fp32 stats) -> xn1 bf16 -> DMA-transpose -> xn1T
  QKV proj (PE; bias rows added via K=1 matmuls into PSUM)
  l2norm(q)*exp(clamped logit_scale), l2norm(k)  (token-major)
  DMA-transpose qn,kn -> feature-major; v kept token-major with ones column
  flash-style attention per head pair: scoresT -> exp (ACT, 2 heads/op)
    -> ctxT + softmax denominator via [v|1] matmul accumulation
  normalize ctx (batched reciprocal + ones-matmul row broadcast),
  out-proj + residual, LN2, MLP (gelu bias fused into ACT), residual -> y.
"""

from contextlib import ExitStack

import numpy as np
import ml_dtypes

import concourse.bass as bass
import concourse.tile as tile
from concourse import bacc, mybir
from concourse.bass import ts, ds
from concourse.bass_utils import run_bass_kernel_spmd

F32 = mybir.dt.float32
F32R = mybir.dt.float32r
BF16 = mybir.dt.bfloat16
AF = mybir.ActivationFunctionType
ALU = mybir.AluOpType

P = 128
B, S, D = 2, 2048, 1024
H, HD = 16, 64
MLP = 4096
SQ = S // 4          # 512 query tokens per core
DC = D // P          # 8
TB = S // P          # 16
TQ = SQ // P         # 4
MC = MLP // P        # 32
HP = H // 2          # 8 head pairs
EPS_LN = 1e-6
EPS_NORM = 1e-12
LOG_MAX = float(np.log(1.0 / 0.01))
N_CORES = 8
SKIP_CC = False

_CACHED_NC = {}


def _emit_once(tc, outs, ins, pools):
    nc = tc.nc

    xq, xqr = ins["xq"], ins["xqr"]
    y = outs["y"]

    # ---- constants ----
    eps_tile = pools["const"].tile([P, 1], F32, tag="eps", name="eps")
    nc.vector.memset(eps_tile[:], EPS_LN)
    eps0 = pools["const"].tile([P, 1], F32, tag="eps0", name="eps0")
    nc.vector.memset(eps0[:], 0.0)
    ones_tok = pools["const"].tile([1, P], BF16, tag="ones_tok", name="ones_tok")
    nc.vector.memset(ones_tok[:], 1.0)
    ones_hd = pools["const"].tile([1, HD], F32, tag="ones_hd", name="ones_hd")  # fp32 for the fp32 bcast matmul
    nc.vector.memset(ones_hd[:], 1.0)

    # bias rows live in DRAM; streamed per-use into [1,512] scratch tiles
    def bias_rhs(name, n):
        rrow = pools["rrow"].tile([1, 512], BF16, tag="rrow", name="rrow")
        nc.sync.dma_start(rrow[:], ins[name][0:1, ts(n, 512)])
        return rrow[:]

    bias_m = pools["const"].tile([P, MC], F32, tag="bias_m", name="bias_m")
    nc.sync.dma_start(bias_m[:], ins["bias_m"][:])

    # per-head scale c = exp(min(logit_scale, LOG_MAX)), broadcast on partitions
    crow = pools["const"].tile([1, H], F32, tag="crow", name="crow")
    nc.sync.dma_start(crow[:], ins["ck"][:])
    c_b = pools["const"].tile([P, H], F32, tag="c_b", name="c_b")
    nc.gpsimd.partition_broadcast(c_b[:], crow[:])

    # ---- persistent activations ----
    xnqT = pools["xnqT"].tile([P, DC, SQ], BF16, tag="xnqT", name="xnqT")   # 1 MB
    knT = pools["knT"].tile([P, DC, S], BF16, tag="knT", name="knT")       # 4 MB
    qnT = pools["qnT"].tile([P, DC, SQ], BF16, tag="qnT", name="qnT")      # 1 MB
    vaug = pools["vaug"].tile([P, TB, H, HD + 1], BF16, tag="vaug", name="vaug")  # 4.25 MB
    ao_dram = pools["dram"].tile([SQ, D], F32, tag="aodram", name="aodram")
    knTo = pools["ctxU"].tile([P, DC, SQ], BF16, tag="ctxU", name="knTo")
    vaugo = pools["ctxU"].tile([P, TQ, H, HD + 1], BF16, tag="btmp", name="vaugo")
    den_halves = [
        pools["den"].tile([H // 2, SQ], F32, tag="den_lo", name="den_lo"),
        pools["den"].tile([H // 2, SQ], F32, tag="den_hi", name="den_hi"),
    ]

    def den_row(h):
        return den_halves[h // (H // 2)][h % (H // 2):h % (H // 2) + 1, :]

    # ones columns of own v-augmented (v evictions later overwrite cols 0:HD)
    nc.vector.memset(vaugo[:], 1.0)

    def ln_tile(x_ap, out_bf16_ap):
        """LayerNorm stats+apply for one [P, D] fp32 tile -> bf16 (gain folded
        into weights on host, ln-bias folded into projection bias rows)."""
        st = pools["stats"].tile([P, 2, 6], F32, tag="st", name="st")
        xr = x_ap.rearrange("p (s d) -> p s d", s=2)
        for i in range(2):
            nc.vector.bn_stats(st[:, i, :], xr[:, i, :])
        mv = pools["stats"].tile([P, 2], F32, tag="mv", name="mv")
        nc.vector.bn_aggr(mv[:], st[:])
        rstd = pools["stats"].tile([P, 1], F32, tag="rstd", name="rstd")
        nc.scalar.activation(rstd[:], mv[:, 1:2], AF.Sqrt, bias=eps_tile[:])
        nc.vector.reciprocal(rstd[:], rstd[:])
        nc.vector.tensor_scalar(out_bf16_ap, x_ap, scalar1=mv[:, 0:1],
                                scalar2=rstd[:], op0=ALU.subtract, op1=ALU.mult)

    # ---- PE warm-up: keep HAM busy while LN1 runs (results unused but kept
    # live via a tiny DRAM spill so DCE keeps them) ----
    wu = pools["const"].tile([P, P], BF16, tag="wu", name="wu")
    nc.vector.memset(wu[:], 0.5)
    wups = pools["mm512"].tile([P, 512], F32, tag="mm512", name="wups")
    for i in range(40):
        nc.tensor.matmul(wups[:, 0:P], wu[:], wu[:],
                         start=(i == 0), stop=(i == 39), skip_group_check=True)
    wusb = pools["const"].tile([P, 4], F32, tag="wusb", name="wusb")
    nc.vector.tensor_copy(wusb[:], wups[:, 0:4])
    wuspill = pools["dram"].tile([P, 4], F32, tag="wuspill", name="wuspill")
    nc.sync.dma_start(wuspill[:], wusb[:])

    # ---- LN1 over own tokens -> xnqT ----
    for t in range(TQ):
        x_t = pools["xin"].tile([P, D], F32, tag="x", name="x")
        nc.sync.dma_start(x_t[:], xq[ts(t, P), :])
        xn_t = pools["xn"].tile([P, D], BF16, tag="xn", name="xn")
        ln_tile(x_t[:], xn_t[:])
        for d in range(DC):
            nc.sync.dma_start(xnqT[:, d, ts(t, P)], xn_t[:, ts(d, P)], transpose=True)

    # ---- QKV projections ----
    def l2norm_scale_transpose(t, kq_t, dstT, scale_pp):
        """kq_t: [P, D] bf16 token-major; optional scale_pp [P, H] extra
        multiplier; writes l2-normalized transpose into dstT[:, :, ts(t, P)]."""
        sq = pools["sq"].tile([P, D], BF16, tag="sq", name="sq")
        nc.scalar.activation(sq[:], kq_t[:], AF.Square)
        ss = pools["stats"].tile([P, H], F32, tag="ss", name="ss")
        nc.vector.tensor_reduce(ss[:], sq[:].rearrange("p (h d) -> p h d", h=H),
                                axis=mybir.AxisListType.X, op=ALU.add)
        nrm = pools["stats"].tile([P, H], F32, tag="nrm", name="nrm")
        nc.scalar.activation(nrm[:], ss[:], AF.Sqrt, bias=eps0[:])
        nc.vector.tensor_scalar_max(nrm[:], nrm[:], EPS_NORM)
        rinv = pools["stats"].tile([P, H], F32, tag="rinv", name="rinv")
        nc.vector.reciprocal(rinv[:], nrm[:])
        if scale_pp is not None:
            nc.vector.tensor_tensor(rinv[:], rinv[:], scale_pp, op=ALU.mult)
        kn_t = pools["xn"].tile([P, D], BF16, tag="xn", name="xn")
        nc.vector.tensor_tensor(
            kn_t[:].rearrange("p (h d) -> p h d", h=H),
            kq_t[:].rearrange("p (h d) -> p h d", h=H),
            rinv[:, :, None].broadcast_to([P, H, HD]), op=ALU.mult)
        for d in range(DC):
            nc.sync.dma_start(dstT[:, d, ts(t, P)], kn_t[:, ts(d, P)], transpose=True)

    qk_tiles = {}

    def evict_q(t, ps):
        q_t = pools["qk"].tile([P, D], BF16, tag="qk", name="qk")
        qk_tiles[t] = q_t
        nc.vector.tensor_copy(q_t[:], ps[:])
        l2norm_scale_transpose(t, q_t, qnT, c_b[:])

    def evict_k(t, ps):
        k_t = pools["qk"].tile([P, D], BF16, tag="qk", name="qk")
        qk_tiles[t] = k_t
        nc.vector.tensor_copy(k_t[:], ps[:])
        l2norm_scale_transpose(t, k_t, knTo, None)

    def evict_v(t, ps):
        nc.vector.tensor_copy(vaugo[:, t, :, 0:HD],
                              ps[:].rearrange("p (h d) -> p h d", h=H))

    def proj(w_name, bias_name, src_T, ntiles, evict):
        w_sb = pools["w"].tile([P, DC, D], BF16, tag="w", name="w")
        nc.sync.dma_start(
            w_sb[:], ins[w_name][:].rearrange("(dc p) c -> p dc c", p=P))
        for t in range(ntiles):
            ps = pools["score"].tile([P, 1024], F32, tag="score", name="psqkv")
            for d in range(DC):
                lhs = src_T[:, d, ts(t, P)]
                nc.tensor.matmul(ps[:, 0:512], lhs, w_sb[:, d, 0:512],
                                 start=(d == 0), stop=False,
                                 skip_group_check=True)
                nc.tensor.matmul(ps[:, 512:1024], lhs, w_sb[:, d, 512:1024],
                                 start=(d == 0), stop=False,
                                 skip_group_check=True)
            for n in range(2):
                nc.tensor.matmul(ps[:, ts(n, 512)], ones_tok[:],
                                 bias_rhs(bias_name, n),
                                 start=False, stop=True, skip_group_check=True)
            evict(t, ps)

    KVK = DC * SQ
    KVV = TQ * H * (HD + 1)
    GROUPS = [[0, 1, 2, 3], [4, 5, 6, 7]]

    # K projection, then its gather starts while V/Q projections run
    proj("wk", "bias_k", xnqT, TQ, evict_k)
    kb = pools["dram"].tile([P, KVK], BF16, tag="kb", name="kb")
    kg = pools["dram"].tile([4, P, KVK], BF16, tag="kg", name="kg")
    nc.sync.dma_start(kb[:], knTo[:].rearrange("p d s -> p (d s)"))
    if SKIP_CC == "none":
        nc.sync.dma_start(kg[0], kb[:])
    elif SKIP_CC:
        for g in range(4):
            nc.sync.dma_start(kg[g], kb[:])
    else:
        nc.gpsimd.collective_compute(
            "AllGather", ALU.bypass, replica_groups=GROUPS,
            ins=[kb[:].opt()], outs=[kg[:].opt()])

    proj("wv", "bias_v", xnqT, TQ, evict_v)
    vb = pools["dram"].tile([P, KVV], BF16, tag="vb", name="vb")
    vg = pools["dram"].tile([4, P, KVV], BF16, tag="vg", name="vg")
    nc.sync.dma_start(vb[:], vaugo[:].rearrange("p t h d -> p (t h d)"))
    if SKIP_CC == "none":
        nc.sync.dma_start(vg[0], vb[:])
    elif SKIP_CC:
        for g in range(4):
            nc.sync.dma_start(vg[g], vb[:])
    else:
        nc.gpsimd.collective_compute(
            "AllGather", ALU.bypass, replica_groups=GROUPS,
            ins=[vb[:].opt()], outs=[vg[:].opt()])

    # q projection runs while the collectives are in flight
    proj("wq", "bias_q", xnqT, TQ, evict_q)
    for g in range(4):
        for d in range(DC):
            nc.sync.dma_start(knT[:, d, ds(SQ * g, SQ)],
                              kg[g, :, ds(512 * d, 512)])
        nc.sync.dma_start(
            vaug[:, ds(TQ * g, TQ), :, :],
            vg[g].rearrange("p (t h d) -> p t h d", t=TQ, h=H))

    # ---- attention: head pairs ----
    ctxU = pools["ctxU"].tile([P, DC, SQ], BF16, tag="ctxU", name="ctxU")
    btmp = pools["ctxU"].tile([HD, HP, SQ], BF16, tag="btmp", name="btmp")
    # softmax denominators: half-batched reciprocal + K=1 broadcast matmul,
    # emitted per 4-head-pair wave so normalization overlaps later attention
    def normalize_heads(h0, h1):
        dh = den_halves[h0 // (H // 2)]
        nc.vector.reciprocal(dh[:], dh[:])
        for h in range(h0, h1):
            hp = h // 2
            rd0 = pools["rd0"].tile([1, SQ], F32, tag="rd0", name="rd0")
            nc.sync.dma_start(rd0[:], den_row(h))
            dn = pools["mm512"].tile([P, 512], F32, tag="mm512", name="mm512")
            nc.tensor.matmul(dn[0:HD, :], ones_hd[:], rd0[:],
                             start=True, stop=True)
            if h % 2 == 0:
                nc.vector.tensor_tensor(ctxU[0:HD, hp, :], ctxU[0:HD, hp, :],
                                        dn[0:HD, :], op=ALU.mult)
            else:
                nc.vector.tensor_tensor(btmp[:, hp, :], btmp[:, hp, :],
                                        dn[0:HD, :], op=ALU.mult)
                nc.sync.dma_start(ctxU[HD:P, hp, :], btmp[:, hp, :])


    for hp in range(HP):
        hA, hB = 2 * hp, 2 * hp + 1
        # alternate psum pools so the next pair's accumulators don't wait on
        # this pair's evictions (mm512 banks are idle during the hp loop)
        cpool = pools["ctx"] if hp % 2 == 0 else pools["mm512"]
        ctag = "ctx" if hp % 2 == 0 else "mm512"
        ctxA = cpool.tile([HD + 1, 512], F32, tag=ctag, name="ctx")
        ctxB = cpool.tile([HD + 1, 512], F32, tag=ctag, name="ctx")
        def emit_scores(kt):
            sc = pools["score"].tile([P, 1024], F32, tag="score", name="score")
            nc.tensor.matmul(sc[:, 0:512], knT[0:HD, hp, ts(kt, P)],
                             qnT[0:HD, hp, :], start=True, stop=True,
                             tile_position=(0, 0), skip_group_check=True)
            nc.tensor.matmul(sc[:, 512:1024], knT[HD:P, hp, ts(kt, P)],
                             qnT[HD:P, hp, :], start=True, stop=True,
                             tile_position=(64, 0), skip_group_check=True)
            return sc

        # software pipeline: kt+1's scores issue on the PE before kt's ctx
        # matmuls, so the in-order PE never stalls waiting for exp(kt)
        sc = emit_scores(0)
        for kt in range(TB):
            eT = pools["eT"].tile([P, 1024], BF16, tag="eT", name="eT")
            nc.scalar.activation(eT[:], sc[:], AF.Exp)
            if kt + 1 < TB:
                sc = emit_scores(kt + 1)
            nc.tensor.matmul(ctxA[:], vaug[:, kt, hA, :], eT[:, 0:512],
                             start=(kt == 0), stop=(kt == TB - 1),
                             skip_group_check=True)
            nc.tensor.matmul(ctxB[:], vaug[:, kt, hB, :], eT[:, 512:1024],
                             start=(kt == 0), stop=(kt == TB - 1),
                             skip_group_check=True)
        # unnormalized evictions + denominator collection
        nc.vector.tensor_copy(ctxU[0:HD, hp, :], ctxA[0:HD, :])
        nc.vector.tensor_copy(btmp[:, hp, :], ctxB[0:HD, :])
        dtmp = pools["dtmp"].tile([HD + 1, 2, 512], F32, tag="dtmp", name="dtmp")
        nc.vector.tensor_copy(dtmp[HD:HD + 1, 0, :], ctxA[HD:HD + 1, :])
        nc.vector.tensor_copy(dtmp[HD:HD + 1, 1, :], ctxB[HD:HD + 1, :])
        nc.sync.dma_start(den_row(hA), dtmp[HD:HD + 1, 0, :])
        nc.sync.dma_start(den_row(hB), dtmp[HD:HD + 1, 1, :])
        if hp == HP // 2 - 1:
            normalize_heads(0, H // 2)
        elif hp == HP - 1:
            normalize_heads(H // 2, H)

    # ---- out-projection + residual -> ao (fp32, token-major) ----
    wo_sb = pools["w"].tile([P, DC, D], BF16, tag="w", name="w")
    nc.sync.dma_start(wo_sb[:], ins["wo"][:].rearrange("(dc p) c -> p dc c", p=P))
    xn2T = pools["xnqT"].tile([P, DC, SQ], BF16, tag="xnqT", name="xn2T")
    for t in range(TQ):
        ao_t = pools["ao"].tile([P, D], F32, tag="ao", name="ao")
        ps = pools["score"].tile([P, 1024], F32, tag="score", name="psao")
        for d in range(DC):
            lhs = ctxU[:, d, ts(t, P)]
            nc.tensor.matmul(ps[:, 0:512], lhs, wo_sb[:, d, 0:512],
                             start=(d == 0), stop=(d == DC - 1),
                             skip_group_check=True)
            nc.tensor.matmul(ps[:, 512:1024], lhs, wo_sb[:, d, 512:1024],
                             start=(d == 0), stop=(d == DC - 1),
                             skip_group_check=True)
        for n in range(2):
            xqr_t = pools["xqr"].tile([P, 512], F32, tag="xqr", name="xqr")
            nc.sync.dma_start(xqr_t[:], xqr[ts(t, P), ts(n, 512)])
            nc.vector.tensor_tensor(ao_t[:, ts(n, 512)], ps[:, ts(n, 512)],
                                    xqr_t[:], op=ALU.add)
        # LN2 for this tile + spill ao to DRAM for the fc2 residual
        xn_t = pools["xn"].tile([P, D], BF16, tag="xn", name="xn")
        ln_tile(ao_t[:], xn_t[:])
        for d in range(DC):
            nc.sync.dma_start(xn2T[:, d, ts(t, P)], xn_t[:, ts(d, P)], transpose=True)
        nc.sync.dma_start(ao_dram[ts(t, P), :], ao_t[:])

    # ---- MLP fc1: h1T feature-major with fused gelu+bias ----
    h1gA = pools["xnT"].tile([P, MC // 2, SQ], BF16, tag="xnTa", name="h1gA")
    h1gB = pools["xnT"].tile([P, MC // 2, SQ], BF16, tag="xnTb", name="h1gB")

    def h1g(m):
        return h1gA[:, m, :] if m < MC // 2 else h1gB[:, m - MC // 2, :]

    for m in range(MC):
        w1_m = pools["w1"].tile([P, DC, P], BF16, tag="w1", name="w1")
        nc.sync.dma_start(
            w1_m[:], ins["w1"][:, ts(m, P)].rearrange("(dc p) c -> p dc c", p=P))
        ps = pools["mm512"].tile([P, 512], F32, tag="mm512", name="mm512")
        for d in range(DC):
            nc.tensor.matmul(ps[:], w1_m[:, d, :], xn2T[:, d, :],
                             start=(d == 0), stop=(d == DC - 1))
        nc.scalar.activation(h1g(m), ps[:], AF.Gelu,
                             bias=bias_m[:, m:m + 1])

    # ---- MLP fc2 + bias + residual -> y ----
    w2_tiles = [
        pools["knT"].tile([P, MC, 512], BF16, tag="knT", name="w2n0"),
        pools["vaug"].tile([P, MC, 512], BF16, tag="vaug", name="w2n1"),
    ]
    for n in range(2):
        w2_n = w2_tiles[n]
        nc.sync.dma_start(
            w2_n[:], ins["w2"][:, ts(n, 512)].rearrange("(mc p) c -> p mc c", p=P))
        for t in range(TQ):
            ps = pools["mm512"].tile([P, 512], F32, tag="mm512", name="mm512")
            for m in range(MC):
                nc.tensor.matmul(ps[:], h1g(m)[:, ts(t, P)], w2_n[:, m, :],
                                 start=(m == 0), stop=False)
            nc.tensor.matmul(ps[:], ones_tok[:], bias_rhs("b2", n),
                             start=False, stop=True)
            aor = pools["xqr"].tile([P, 512], F32, tag="xqr", name="aor")
            nc.sync.dma_start(aor[:], ao_dram[ts(t, P), ts(n, 512)])
            y_t = pools["yout"].tile([P, 512], F32, tag="yout", name="yout")
            nc.vector.tensor_tensor(y_t[:], ps[:], aor[:], op=ALU.add)
            nc.sync.dma_start(y[ts(t, P), ts(n, 512)], y_t[:])


def build_program(repeat=1, skip_cc=False):
    global SKIP_CC
    SKIP_CC = skip_cc
    nc = bacc.Bacc("TRN2", target_bir_lowering=False, debug=False)
    ins = {}

    def din(name, shape, dt=F32):
        ins[name] = nc.dram_tensor(name, list(shape), dt, kind="ExternalInput").ap()

    din("xq", [SQ, D]); din("xqr", [SQ, D])
    din("wq", [D, D], BF16); din("wk", [D, D], BF16); din("wv", [D, D], BF16)
    din("wo", [D, D], BF16)
    din("w1", [D, MLP], BF16); din("w2", [MLP, D], BF16)
    din("bias_q", [1, D], BF16); din("bias_k", [1, D], BF16)
    din("bias_v", [1, D], BF16)
    din("bias_m", [P, MC]); din("b2", [1, D], BF16); din("ck", [1, H])
    outs = {"y": nc.dram_tensor("y", [SQ, D], F32, kind="ExternalOutput").ap()}

    with tile.TileContext(nc) as tc:
        with ExitStack() as es:
            pools = {}

            def pool(name, bufs, space="SBUF"):
                pools[name] = es.enter_context(
                    tc.tile_pool(name=name, bufs=bufs, space=space))

            pool("const", 1)
            pool("xnT", 1); pool("xnqT", 1); pool("knT", 1); pool("qnT", 1)
            pool("vaug", 1); pool("xqr", 2); pool("ao", 2); pool("ctxU", 1)
            pool("den", 1); pool("rd0", 1); pool("dtmp", 1); pool("rrow", 2)
            pool("xin", 2); pool("xn", 3); pool("stats", 4); pool("sq", 1)
            pool("qk", 2); pool("w", 1); pool("w1", 2)
            pool("eT", 3); pool("yout", 1)
            pool("dram", 1, space="DRAM")
            pool("mm512", 2, space="PSUM")
            pool("score", 2, space="PSUM")
            pool("ctx", 2, space="PSUM")
            for _ in range(repeat):
                _emit_once(tc, outs, ins, pools)
    nc.compile()
    return nc


def _host_prep(inputs):
    """Host-side slicing + folding. Returns per-core in_maps."""
    f32 = np.float32
    bf16 = ml_dtypes.bfloat16
    x = np.asarray(inputs["x"], f32)
    ln1_g = np.asarray(inputs["ln1_g"], f32); ln1_b = np.asarray(inputs["ln1_b"], f32)
    ln2_g = np.asarray(inputs["ln2_g"], f32); ln2_b = np.asarray(inputs["ln2_b"], f32)
    wq = np.asarray(inputs["wq"], f32); wk = np.asarray(inputs["wk"], f32)
    wv = np.asarray(inputs["wv"], f32); wo = np.asarray(inputs["wo"], f32)
    w1 = np.asarray(inputs["w1"], f32); w2 = np.asarray(inputs["w2"], f32)
    bq = np.asarray(inputs["bq"], f32); bk = np.asarray(inputs["bk"], f32)
    bv = np.asarray(inputs["bv"], f32); bo = np.asarray(inputs["bo"], f32)
    b1 = np.asarray(inputs["b1"], f32); b2 = np.asarray(inputs["b2"], f32)
    ls = np.asarray(inputs["logit_scale"], f32).reshape(H)

    shared = dict(
        wq=(ln1_g[:, None] * wq).astype(bf16),
        wk=(ln1_g[:, None] * wk).astype(bf16),
        wv=(ln1_g[:, None] * wv).astype(bf16),
        wo=wo.astype(bf16),
        w1=(ln2_g[:, None] * w1).astype(bf16),
        w2=w2.astype(bf16),
        bias_q=(ln1_b @ wq + bq).astype(bf16).reshape(1, D),
        bias_k=(ln1_b @ wk + bk).astype(bf16).reshape(1, D),
        bias_v=(ln1_b @ wv + bv).astype(bf16).reshape(1, D),
        bias_m=(ln2_b @ w1 + b1).astype(f32).reshape(MC, P).T.copy(),
        b2=b2.astype(bf16).reshape(1, D),
        ck=np.exp(np.minimum(ls, LOG_MAX)).astype(f32).reshape(1, H),
    )
    in_maps = []
    for c in range(N_CORES):
        b = c // 4
        t = c % 4
        sl = slice(t * SQ, (t + 1) * SQ)
        m = dict(shared)
        m["xq"] = np.ascontiguousarray(x[b, sl])
        m["xqr"] = np.ascontiguousarray(x[b, sl] + bo[None, :])
        in_maps.append(m)
    return in_maps


def kernel(**inputs):
    if "main" not in _CACHED_NC:
        _CACHED_NC["main"] = build_program()
    nc = _CACHED_NC["main"]
    in_maps = _host_prep(inputs)
    res = run_bass_kernel_spmd(nc, in_maps, core_ids=list(range(N_CORES)))
    y = np.empty((B, S, D), np.float32)
    for c in range(N_CORES):
        b = c // 4
        t = c % 4
        y[b, t * SQ:(t + 1) * SQ] = res.results[c]["y"]
    return y

